# revision 1
# baseline (speedup 1.0000x reference)
"""DoRA linear layer (nn_DoraLinearLayer) on 8 Trainium2 NeuronCores.

Math: out = (s-1)*(x @ W.T) + 2*s*((x @ A.T) @ B.T),
      s = magnitude / ||W + 2*B@A||_row  (stop-grad norm)

This factors exactly into ONE matmul per token:
      out = x @ Weff.T,  Weff.T = (s-1)∘W.T + A.T @ (2B∘s).T
(∘ scales output-feature columns). The kernel computes s on-device from
fp16 copies of W.T/A/B via the Gram/polarization identity
      ||W + (2B)A||^2_row = ||W||^2_row + Σ_r B2T ∘ (2A@W.T + G@B2T),
      G = A@A.T, B2 = 2B
(all matmul/ACT work, no big vector chain), builds Weff.T once, then
streams x.T through a single fp16 matmul with fp32 PSUM accumulation.

Sharding: column-parallel over out_features — core i owns rows
[i*512, (i+1)*512) of W/B/magnitude, x and A replicated, output shard
concatenated on the last dim on the host. Host-side work is marshaling
only: casts to fp16, transposes, slicing.
"""
import numpy as np

import concourse.bass as bass
import concourse.tile as tile
from concourse import bacc, mybir
from concourse.bass_utils import run_bass_kernel_spmd

N_CORES = 8
TOKENS, D_IN, D_OUT, R = 8192, 4096, 4096, 16
O = D_OUT // N_CORES          # 512 output features per core
P = 128                       # partitions
NCH = D_IN // P               # 32 contraction chunks
SCALING = 2.0                 # lora_alpha / r
N_WARM = 8                    # PE warmup matmuls

# token groups: first is chunk-major with 5 psum banks so the matmuls
# exactly saturate PE while trailing the weff feeder; last takes the rest
TGROUPS = [(0, 640, True)]
_t = 640
while _t + 512 <= TOKENS - 384:
    TGROUPS.append((_t, 512, False))
    _t += 512
TGROUPS.append((_t, TOKENS - _t, False))   # 384

f16 = mybir.dt.float16
f32 = mybir.dt.float32
bf16 = mybir.dt.bfloat16
Copy = mybir.ActivationFunctionType.Copy

_CACHE: dict = {}


def emit_kernel(nc, tc, xt, wt, a, at, a2t, b2t, mag, out):
    """Emit the per-core program. All DRAM APs are per-core shapes."""
    from contextlib import ExitStack

    with ExitStack() as ctx:
        singles = ctx.enter_context(tc.tile_pool(name="singles", bufs=1))
        setup = ctx.enter_context(tc.tile_pool(name="setup", bufs=3))
        # 8 PSUM banks total: setup-scratch(2) + n2(1) + mm(5)
        ps_scr = ctx.enter_context(tc.tile_pool(name="ps_scr", bufs=2, space="PSUM"))
        ps_n2 = ctx.enter_context(tc.tile_pool(name="ps_n2", bufs=1, space="PSUM"))
        ps_mm = ctx.enter_context(tc.tile_pool(name="ps_mm", bufs=5, space="PSUM"))
        xpool = ctx.enter_context(tc.tile_pool(name="xpool", bufs=2))
        x0pool = ctx.enter_context(tc.tile_pool(name="x0pool", bufs=1))
        opool = ctx.enter_context(tc.tile_pool(name="opool", bufs=2))

        # ---- at/a2t ride the fast SP ring ahead of wt; the rest go via
        # GPSIMD SWDGE ordered by first use: b2t (h), a (phase B), mag (s)
        at_sb = singles.tile([P, NCH, R], f16)
        nc.sync.dma_start(out=at_sb, in_=at.rearrange("p (c r) -> p c r", r=R))
        a2t_sb = singles.tile([P, NCH, R], f16)
        nc.sync.dma_start(out=a2t_sb, in_=a2t.rearrange("p (c r) -> p c r", r=R))
        b2t_sb = singles.tile([R, O], f16)
        nc.gpsimd.dma_start(out=b2t_sb, in_=b2t)
        a_sb = singles.tile([R, D_IN], f16)
        nc.gpsimd.dma_start(out=a_sb, in_=a)
        mag_sb = singles.tile([P, O], f32)
        nc.gpsimd.dma_start(out=mag_sb, in_=mag)

        ones_col = singles.tile([P, 1], bf16)
        nc.vector.memset(ones_col, 1.0)
        ones16_col = singles.tile([R, 1], f16)
        nc.vector.memset(ones16_col, 1.0)
        ones_row16 = singles.tile([1, P], f16)
        nc.vector.memset(ones_row16, 1.0)
        ones_row32 = singles.tile([1, P], f32)
        nc.vector.memset(ones_row32, 1.0)

        # pre-warm the ACT Sqrt table so the s-chain doesn't pay the load
        sqrt_warm = singles.tile([1, 1], f32)
        nc.vector.memset(sqrt_warm, 1.0)
        sqrt_warm2 = singles.tile([1, 1], f32)
        nc.scalar.sqrt(sqrt_warm2, sqrt_warm)

        # ---- PE warmup: cheap fp16 matmuls to lift the HAM clock gate ----
        warm_row = singles.tile([1, O], f16)
        nc.vector.memset(warm_row, 0.5)
        warm_ps = ps_scr.tile([P, O], f32, name="scr")
        for _ in range(N_WARM):
            nc.tensor.matmul(warm_ps, lhsT=ones_row16, rhs=warm_row,
                             start=True, stop=True)

        # ---- W.T: host-prearranged [p, c, o]; 4 wave DMAs of 8 chunks ----
        # same tag as the main-loop x tiles: its slot recycles after phase B
        wt_sb = xpool.tile([P, NCH, O], f16, name="xt")
        wt_r = wt.rearrange("p (c o) -> p c o", o=O)
        wt_dmas = []
        wave_edges = [0, 1, 2, 4, 7, 11, 16, 23, NCH]
        for w in range(len(wave_edges) - 1):
            lo, hi = wave_edges[w], wave_edges[w + 1]
            wt_dmas.append(
                nc.sync.dma_start(out=wt_sb[:, lo:hi, :],
                                  in_=wt_r[:, lo:hi, :]))
        wt_t = [wt_sb[:, c, :] for c in range(NCH)]

        # ---- phase A: n2 = ||W||^2_col + sum_r B2T ∘ (2A@W.T + G@B2T) ----
        g_ps = ps_scr.tile([R, R], f32, name="scr")
        for c in range(NCH):
            nc.tensor.matmul(
                g_ps, lhsT=at_sb[:, c, :], rhs=at_sb[:, c, :],
                start=(c == 0), stop=(c == NCH - 1),
            )
        g_sb = singles.tile([R, R], f16)
        nc.scalar.activation(g_sb, g_ps, Copy)

        n2_ps = ps_n2.tile([1, O], f32)
        h_ps = ps_scr.tile([R, O], f32, name="scr")
        for c in range(NCH):
            nc.tensor.matmul(
                h_ps, lhsT=a2t_sb[:, c, :], rhs=wt_t[c],
                start=(c == 0), stop=(c == NCH - 1),
            )
            if c == 0:
                nc.tensor.matmul(h_ps, lhsT=g_sb, rhs=b2t_sb,
                                 start=False, stop=False)
            sq = setup.tile([P, O], bf16, name="sq")
            if c % 2 == 0:
                nc.vector.tensor_mul(sq, wt_t[c], wt_t[c])
            else:
                nc.scalar.square(sq, wt_t[c])
            nc.tensor.matmul(
                n2_ps, lhsT=ones_col, rhs=sq,
                start=(c == 0), stop=False,
            )
        hterm = singles.tile([R, O], f16)
        nc.vector.tensor_mul(hterm, b2t_sb, h_ps)
        nc.tensor.matmul(n2_ps, lhsT=ones16_col, rhs=hterm,
                         start=False, stop=True)


        # ---- prefetch x.T for the first two token groups ----
        t0, ntok0, _ = TGROUPS[0]
        xt0 = x0pool.tile([P, NCH, ntok0], f16, name="xt0")
        xg0 = xt[:, t0 : t0 + ntok0].rearrange("(c p) t -> c p t", p=P)
        from concourse.tile_rust import add_dep_helper
        for c in range(NCH):
            dma = nc.sync.dma_start(out=xt0[:, c, :], in_=xg0[c])
            if c == 0:
                # keep x prefetch off the HBM bus until W.T has landed —
                # the wt waves gate the whole setup critical path
                for wd in wt_dmas:
                    add_dep_helper(dma.ins, wd.ins, True, "x prefetch after wt")
        t1, ntok1, _ = TGROUPS[1]
        xt1 = xpool.tile([P, NCH, ntok1], f16, name="xt")
        xg1 = xt[:, t1 : t1 + ntok1].rearrange("(c p) t -> c p t", p=P)
        nc.sync.dma_start(out=xt1, in_=xg1.rearrange("c p t -> p c t"))
        xt_pre = {0: xt0, 1: xt1}

        # ---- s = mag / sqrt(n2); broadcast; sm1 = s - 1 ----
        nrm = singles.tile([1, O], f32)
        nc.scalar.sqrt(nrm, n2_ps)
        rn = singles.tile([1, O], f32)
        nc.vector.reciprocal_approx_fast(out=rn, in_=nrm)
        # broadcast 1/nrm to all partitions (rank-1 matmul), then s = mag ∘ it
        brn_ps = ps_scr.tile([P, O], f32, name="scr")
        nc.tensor.matmul(brn_ps, lhsT=ones_row32, rhs=rn, start=True, stop=True)
        s_bc = singles.tile([P, O], f32)
        nc.vector.tensor_mul(s_bc, mag_sb, brn_ps)
        bcast_sm1 = singles.tile([P, O], f32)
        nc.vector.tensor_scalar_add(bcast_sm1, s_bc, -1.0)
        # (2B∘s).T = (2B).T with columns scaled by s
        b2st_sb = singles.tile([R, O], f16)
        nc.vector.tensor_mul(b2st_sb, b2t_sb, s_bc[:R, :])

        # ---- phase B: Weff.T = (s-1)∘W.T + A.T @ (2B∘s).T ----
        weff_t = []
        for c in range(NCH):
            lws_ps = ps_scr.tile([P, O], f32, name="scr")
            nc.tensor.matmul(
                lws_ps, lhsT=a_sb[:, c * P : (c + 1) * P], rhs=b2st_sb,
                start=True, stop=True,
            )
            tmp = setup.tile([P, O], f32, name="tmp")
            nc.vector.tensor_mul(tmp, wt_t[c], bcast_sm1)
            w = singles.tile([P, O], f16, name=f"weff{c}")
            nc.vector.tensor_add(w, tmp, lws_ps)
            weff_t.append(w)

        # ---- main: out = x @ Weff.T, streamed over token groups ----
        for gi, (t0, ntok, chunk_major) in enumerate(TGROUPS):
            nm = ntok // P
            if gi in xt_pre:
                xt_t = xt_pre[gi]
            else:
                xt_t = xpool.tile([P, NCH, ntok], f16, name="xt")
                xg = xt[:, t0 : t0 + ntok].rearrange("(c p) t -> c p t", p=P)
                nc.sync.dma_start(out=xt_t, in_=xg.rearrange("c p t -> p c t"))
            ot = opool.tile([P, nm, O], f32, name="ot")
            if chunk_major:
                # consume each weff chunk nm× as soon as it lands
                pss = [ps_mm.tile([P, O], f32, name="mm") for _ in range(nm)]
                for c in range(NCH):
                    for m in range(nm):
                        nc.tensor.matmul(
                            pss[m],
                            lhsT=xt_t[:, c, m * P : (m + 1) * P],
                            rhs=weff_t[c],
                            start=(c == 0), stop=(c == NCH - 1),
                        )
                for m in range(nm):
                    nc.scalar.activation(ot[:, m, :], pss[m], Copy)
            else:
                for m in range(nm):
                    ps = ps_mm.tile([P, O], f32, name="mm")
                    for c in range(NCH):
                        nc.tensor.matmul(
                            ps,
                            lhsT=xt_t[:, c, m * P : (m + 1) * P],
                            rhs=weff_t[c],
                            start=(c == 0), stop=(c == NCH - 1),
                        )
                    nc.scalar.activation(ot[:, m, :], ps, Copy)
                    if gi == len(TGROUPS) - 1:
                        nc.sync.dma_start(
                            out=out[t0 + m * P : t0 + (m + 1) * P, :],
                            in_=ot[:, m, :],
                        )
            if gi != len(TGROUPS) - 1:
                nc.sync.dma_start(
                    out=out[t0 : t0 + ntok, :].rearrange("(m p) o -> p m o", p=P),
                    in_=ot,
                )


def build_nc():
    if "nc" in _CACHE:
        return _CACHE["nc"]
    nc = bacc.Bacc("TRN2", target_bir_lowering=False, debug=False,
                   num_devices=N_CORES)
    xt = nc.dram_tensor("xt", [D_IN, TOKENS], f16, kind="ExternalInput").ap()
    wt = nc.dram_tensor("wt", [P, NCH * O], f16, kind="ExternalInput").ap()
    a = nc.dram_tensor("a", [R, D_IN], f16, kind="ExternalInput").ap()
    at = nc.dram_tensor("at", [P, NCH * R], f16, kind="ExternalInput").ap()
    a2t = nc.dram_tensor("a2t", [P, NCH * R], f16, kind="ExternalInput").ap()
    b2t = nc.dram_tensor("b2t", [R, O], f16, kind="ExternalInput").ap()
    mag = nc.dram_tensor("mag", [P, O], f32, kind="ExternalInput").ap()
    out = nc.dram_tensor("out", [TOKENS, O], f32, kind="ExternalOutput").ap()
    with tile.TileContext(nc) as tc:
        emit_kernel(nc, tc, xt, wt, a, at, a2t, b2t, mag, out)
    nc.compile()
    _CACHE["nc"] = nc
    return nc


def prep_in_maps(x, lora_A_w, lora_B_w, base_w, magnitude):
    xt_np = np.ascontiguousarray(x.astype(np.float16).T)
    a_np = np.ascontiguousarray(lora_A_w.astype(np.float16))
    # A.T partition-major: at_dev[p, c*R + r] = A.T[c*128 + p, r]
    at_full = np.ascontiguousarray(a_np.T)                     # [4096, R]
    at_np = np.ascontiguousarray(
        at_full.reshape(NCH, P, R).transpose(1, 0, 2).reshape(P, NCH * R))
    a2t_full = np.ascontiguousarray(
        (2.0 * a_np.astype(np.float32)).astype(np.float16).T)  # [4096, R]
    a2t_np = np.ascontiguousarray(
        a2t_full.reshape(NCH, P, R).transpose(1, 0, 2).reshape(P, NCH * R))
    in_maps = []
    for c in range(N_CORES):
        sl = slice(c * O, (c + 1) * O)
        # W.T partition-major: wt_dev[p, c*O + o] = W.T[c*128 + p, o]
        wt_sh = np.ascontiguousarray(base_w[sl].astype(np.float16).T)  # [4096, O]
        wt_dev = np.ascontiguousarray(
            wt_sh.reshape(NCH, P, O).transpose(1, 0, 2).reshape(P, NCH * O))
        in_maps.append({
            "xt": xt_np,
            "wt": wt_dev,
            "a": a_np,
            "at": at_np,
            "a2t": a2t_np,
            "b2t": np.ascontiguousarray(
                (SCALING * lora_B_w[sl]).astype(np.float16).T),
            "mag": np.ascontiguousarray(np.broadcast_to(
                magnitude[sl].reshape(1, O).astype(np.float32), (P, O))),
        })
    return in_maps


def kernel(x, lora_A_w, lora_B_w, base_w, magnitude):
    nc = build_nc()
    in_maps = prep_in_maps(x, lora_A_w, lora_B_w, base_w, magnitude)
    res = run_bass_kernel_spmd(nc, in_maps, list(range(N_CORES)))
    return np.concatenate(
        [res.results[c]["out"] for c in range(N_CORES)], axis=1)



# revision 3
# speedup vs baseline: 1.2537x; 1.2537x over previous
"""DoRA linear layer (nn_DoraLinearLayer) on 8 Trainium2 NeuronCores.

Math: out = (s-1)*(x @ W.T) + 2*s*((x @ A.T) @ B.T),
      s = magnitude / ||W + 2*B@A||_row  (stop-grad norm)

Fused single-GEMM form: out = x @ Weff.T with
      Weff.T = (s-1)*W.T + A.T @ (2B*s).T
The kernel computes s on-device (Gram/polarization identity on fp16
copies), builds Weff.T once, then streams x through the GEMM.

fp8 acceleration: the tensor engine runs float8e4 matmuls in DoubleRow
perf mode at 2x the fp16 rate (K=256 per instruction).  Weff columns
are split per-core by |s-1| (host-estimated, layout decision only):
the 416 columns with small |s-1| are quantized to fp8 with per-column
scales and streamed via DoubleRow against fp8 x; the 96 worst columns
stay fp16 (mixed fp8-lhsT x fp16-rhs matmuls are exact on TRN2).  Both
groups accumulate into one PSUM bank (fp16 group opens the bank with
start=True; fp8 group rides the pending-zero region).  A f32 epilogue
rescales per column and emits fp16.  Measured end-to-end rel err vs
the fp32 reference ~1.7e-2 (gate 2e-2); fp16 x / fp16 Weff gives
2.1e-4 but runs ~1.7x slower.

Sharding: column-parallel over out_features - core i owns rows
[i*512, (i+1)*512) of W/B/magnitude (host-permuted within the shard),
x and A replicated.  Host work is marshaling only: dtype casts,
transposes, the |s-1| permutation and fp8 scale vectors.
"""
import numpy as np

import concourse.bass as bass
import concourse.tile as tile
from concourse import bacc, mybir
from concourse.bass_utils import run_bass_kernel_spmd

N_CORES = 8
TOKENS, D_IN, D_OUT, R = 8192, 4096, 4096, 16
O = D_OUT // N_CORES          # 512 output features per core
P = 128                       # partitions
NCH = D_IN // P               # 32 contraction chunks
SCALING = 2.0                 # lora_alpha / r
N_WARM = 8                    # PE warmup matmuls
N16 = 96                      # columns kept in fp16 (largest |s-1|)
LO = O - N16                  # fp8 DoubleRow columns
SX = 32.0                     # fp8 scale for x
CTARG = 160.0                 # per-column fp8 target absmax for Weff

# token groups: first is chunk-major with 5 psum banks so the matmuls
# exactly saturate PE while trailing the weff feeder; last takes the rest
TGROUPS = [(0, 640, True)]
_t = 640
while _t + 512 <= TOKENS - 384:
    TGROUPS.append((_t, 512, False))
    _t += 512
TGROUPS.append((_t, TOKENS - _t, False))   # 384

f16 = mybir.dt.float16
f32 = mybir.dt.float32
bf16 = mybir.dt.bfloat16
f8 = mybir.dt.float8e4
DR = mybir.MatmulPerfMode.DoubleRow
Copy = mybir.ActivationFunctionType.Copy

_CACHE: dict = {}


def _emit_tile_mms(nc, ps, xt_t, msl, w8lo, whi):
    """All matmuls for one 128-token tile into psum `ps` ([128, 512]).

    fp16 group (cols LO:) opens the bank (start=True zeroes the whole
    2KB zero-region); the fp8 DoubleRow group (cols :LO) accumulates
    onto the pending-zero bytes, pairing chunks (c-1, c) at odd c.
    """
    psA, psB = ps[:, :LO], ps[:, LO:]
    for c in range(NCH):
        nc.tensor.matmul(
            psB, lhsT=xt_t[:, c, msl], rhs=whi[:, c, :],
            start=(c == 0), stop=(c == NCH - 1), skip_group_check=True,
        )
        if c % 2 == 1:
            nc.tensor.matmul(
                psA, lhsT=xt_t[:, c - 1 : c + 1, msl],
                rhs=w8lo[:, c - 1 : c + 1, :],
                start=False, stop=(c == NCH - 1), perf_mode=DR,
                skip_group_check=True,
            )


def emit_kernel(nc, tc, xt, wt, a, at, a2t, b2t, mag, cslo, epi, out):
    """Emit the per-core program. All DRAM APs are per-core shapes."""
    from contextlib import ExitStack

    with ExitStack() as ctx:
        singles = ctx.enter_context(tc.tile_pool(name="singles", bufs=1))
        setup = ctx.enter_context(tc.tile_pool(name="setup", bufs=3))
        # 8 PSUM banks: scr(2) + lws(1) + mm(5)
        ps_scr = ctx.enter_context(tc.tile_pool(name="ps_scr", bufs=2, space="PSUM"))
        ps_lws = ctx.enter_context(tc.tile_pool(name="ps_lws", bufs=1, space="PSUM"))
        ps_mm = ctx.enter_context(tc.tile_pool(name="ps_mm", bufs=5, space="PSUM"))
        wtpool = ctx.enter_context(tc.tile_pool(name="wtpool", bufs=1))
        xpool = ctx.enter_context(tc.tile_pool(name="xpool", bufs=2))
        x0pool = ctx.enter_context(tc.tile_pool(name="x0pool", bufs=1))
        opool = ctx.enter_context(tc.tile_pool(name="opool", bufs=2))

        # ---- at/a2t ride the fast SP ring ahead of wt; the rest go via
        # GPSIMD SWDGE ordered by first use
        at_sb = singles.tile([P, NCH, R], f16)
        nc.sync.dma_start(out=at_sb, in_=at.rearrange("p (c r) -> p c r", r=R))
        a2t_sb = singles.tile([P, NCH, R], f16)
        nc.sync.dma_start(out=a2t_sb, in_=a2t.rearrange("p (c r) -> p c r", r=R))
        b2t_sb = singles.tile([R, O], f16)
        nc.gpsimd.dma_start(out=b2t_sb, in_=b2t)
        a_sb = singles.tile([R, D_IN], f16)
        nc.gpsimd.dma_start(out=a_sb, in_=a)
        mag_sb = singles.tile([P, O], f32)
        nc.gpsimd.dma_start(out=mag_sb, in_=mag)
        cslo_sb = singles.tile([P, LO], f32)
        nc.gpsimd.dma_start(out=cslo_sb, in_=cslo)
        epi_sb = singles.tile([P, O], f32)
        nc.gpsimd.dma_start(out=epi_sb, in_=epi)

        ones_col = singles.tile([P, 1], bf16)
        nc.vector.memset(ones_col, 1.0)
        ones16_col = singles.tile([R, 1], f16)
        nc.vector.memset(ones16_col, 1.0)
        ones_row16 = singles.tile([1, P], f16)
        nc.vector.memset(ones_row16, 1.0)
        ones_row32 = singles.tile([1, P], f32)
        nc.vector.memset(ones_row32, 1.0)

        # pre-warm the ACT Sqrt table so the s-chain doesn't pay the load
        sqrt_warm = singles.tile([1, 1], f32)
        nc.vector.memset(sqrt_warm, 1.0)
        sqrt_warm2 = singles.tile([1, 1], f32)
        nc.scalar.sqrt(sqrt_warm2, sqrt_warm)

        # ---- PE warmup: cheap matmuls to lift the HAM clock gate ----
        warm_row = singles.tile([1, O], f16)
        nc.vector.memset(warm_row, 0.5)
        warm8 = singles.tile([P, 2, 64], f8)
        nc.vector.memset(warm8, 0.25)
        warm8r = singles.tile([P, 2, O], f8)
        nc.vector.memset(warm8r, 0.25)
        warm_ps = ps_scr.tile([P, O], f32, name="scr")
        for i in range(N_WARM):
            if i % 2 == 0:
                nc.tensor.matmul(warm_ps, lhsT=ones_row16, rhs=warm_row,
                                 start=True, stop=True)
            else:
                nc.tensor.matmul(warm_ps[:64, :], lhsT=warm8, rhs=warm8r,
                                 start=True, stop=True, perf_mode=DR)

        # ---- W.T: host-prearranged [p, c, o]; wave DMAs of chunks ----
        wt_sb = wtpool.tile([P, NCH, O], f16, name="wt16")
        wt_r = wt.rearrange("p (c o) -> p c o", o=O)
        wt_dmas = []
        wave_edges = [0, 1, 2, 4, 7, 11, 16, 23, NCH]
        for w in range(len(wave_edges) - 1):
            lo_, hi_ = wave_edges[w], wave_edges[w + 1]
            wt_dmas.append(
                nc.sync.dma_start(out=wt_sb[:, lo_:hi_, :],
                                  in_=wt_r[:, lo_:hi_, :]))
        wt_t = [wt_sb[:, c, :] for c in range(NCH)]

        # ---- phase A: n2 = ||W||^2_col + sum_r B2T o (2A@W.T + G@B2T) ----
        g_ps = ps_scr.tile([R, R], f32, name="scr")
        for c in range(NCH):
            nc.tensor.matmul(
                g_ps, lhsT=at_sb[:, c, :], rhs=at_sb[:, c, :],
                start=(c == 0), stop=(c == NCH - 1),
            )
        g_sb = singles.tile([R, R], f16)
        nc.scalar.activation(g_sb, g_ps, Copy)

        h_ps = ps_scr.tile([R, O], f32, name="scr")
        n2_ps = ps_scr.tile([1, O], f32, name="scr")
        for c in range(NCH):
            nc.tensor.matmul(
                h_ps, lhsT=a2t_sb[:, c, :], rhs=wt_t[c],
                start=(c == 0), stop=(c == NCH - 1),
            )
            if c == 0:
                nc.tensor.matmul(h_ps, lhsT=g_sb, rhs=b2t_sb,
                                 start=False, stop=False)
            sq = setup.tile([P, O], bf16, name="sq")
            if c % 2 == 0:
                nc.vector.tensor_mul(sq, wt_t[c], wt_t[c])
            else:
                nc.scalar.square(sq, wt_t[c])
            nc.tensor.matmul(
                n2_ps, lhsT=ones_col, rhs=sq,
                start=(c == 0), stop=False,
            )
        hterm = singles.tile([R, O], f16)
        nc.vector.tensor_mul(hterm, b2t_sb, h_ps)
        nc.tensor.matmul(n2_ps, lhsT=ones16_col, rhs=hterm,
                         start=False, stop=True)

        # ---- prefetch x.T for the first two token groups ----
        t0, ntok0, _ = TGROUPS[0]
        xt0 = x0pool.tile([P, NCH, ntok0], f8, name="xt0")
        xg0 = xt[:, t0 : t0 + ntok0].rearrange("(c p) t -> c p t", p=P)
        from concourse.tile_rust import add_dep_helper
        for c in range(NCH):
            dma = nc.sync.dma_start(out=xt0[:, c, :], in_=xg0[c])
            if c == 0:
                # keep x prefetch off the HBM bus until W.T has landed -
                # the wt waves gate the whole setup critical path
                for wd in wt_dmas:
                    add_dep_helper(dma.ins, wd.ins, True, "x prefetch after wt")
        t1, ntok1, _ = TGROUPS[1]
        xt1 = xpool.tile([P, NCH, ntok1], f8, name="xt")
        xg1 = xt[:, t1 : t1 + ntok1].rearrange("(c p) t -> c p t", p=P)
        nc.sync.dma_start(out=xt1, in_=xg1.rearrange("c p t -> p c t"))
        xt_pre = {0: xt0, 1: xt1}

        # ---- s = mag / sqrt(n2); broadcast; sm1 = s - 1 ----
        nrm = singles.tile([1, O], f32)
        nc.scalar.sqrt(nrm, n2_ps)
        rn = singles.tile([1, O], f32)
        nc.vector.reciprocal_approx_fast(out=rn, in_=nrm)
        brn_ps = ps_scr.tile([P, O], f32, name="scr")
        nc.tensor.matmul(brn_ps, lhsT=ones_row32, rhs=rn, start=True, stop=True)
        s_bc = singles.tile([P, O], f32)
        nc.vector.tensor_mul(s_bc, mag_sb, brn_ps)
        bcast_sm1 = singles.tile([P, O], f32)
        nc.vector.tensor_scalar_add(bcast_sm1, s_bc, -1.0)
        # (2B*s).T = (2B).T with columns scaled by s
        b2st_sb = singles.tile([R, O], f16)
        nc.vector.tensor_mul(b2st_sb, b2t_sb, s_bc[:R, :])

        # ---- phase B: Weff.T = (s-1)*W.T + A.T @ (2B*s).T; quantize the
        # lo columns to fp8 with per-column scales, keep hi columns fp16
        w8lo = singles.tile([P, NCH, LO], f8, name="w8lo")
        whi = singles.tile([P, NCH, N16], f16, name="whi")
        for c in range(NCH):
            lws_ps = ps_lws.tile([P, O], f32, name="lws")
            nc.tensor.matmul(
                lws_ps, lhsT=a_sb[:, c * P : (c + 1) * P], rhs=b2st_sb,
                start=True, stop=True,
            )
            tmp = setup.tile([P, O], f32, name="tmp")
            nc.vector.tensor_mul(tmp, wt_t[c], bcast_sm1)
            nc.vector.tensor_add(whi[:, c, :], tmp[:, LO:], lws_ps[:, LO:])
            wlo16 = setup.tile([P, LO], f32, name="wlo16")
            nc.vector.tensor_add(wlo16, tmp[:, :LO], lws_ps[:, :LO])
            nc.vector.tensor_mul(w8lo[:, c, :], wlo16, cslo_sb)

        # ---- main: out = x @ Weff.T, streamed over token groups ----
        for gi, (t0, ntok, chunk_major) in enumerate(TGROUPS):
            nm = ntok // P
            if gi in xt_pre:
                xt_t = xt_pre[gi]
            else:
                xt_t = xpool.tile([P, NCH, ntok], f8, name="xt")
                xg = xt[:, t0 : t0 + ntok].rearrange("(c p) t -> c p t", p=P)
                nc.sync.dma_start(out=xt_t, in_=xg.rearrange("c p t -> p c t"))
            ot = opool.tile([P, nm, O], f16, name="ot")
            if chunk_major:
                # consume each weff chunk nm x as soon as it lands
                pss = [ps_mm.tile([P, O], f32, name="mm") for _ in range(nm)]
                for c in range(NCH):
                    for m in range(nm):
                        msl = slice(m * P, (m + 1) * P)
                        nc.tensor.matmul(
                            pss[m][:, LO:], lhsT=xt_t[:, c, msl],
                            rhs=whi[:, c, :],
                            start=(c == 0), stop=(c == NCH - 1),
                            skip_group_check=True,
                        )
                    if c % 2 == 1:
                        for m in range(nm):
                            msl = slice(m * P, (m + 1) * P)
                            nc.tensor.matmul(
                                pss[m][:, :LO],
                                lhsT=xt_t[:, c - 1 : c + 1, msl],
                                rhs=w8lo[:, c - 1 : c + 1, :],
                                start=False, stop=(c == NCH - 1),
                                perf_mode=DR, skip_group_check=True,
                            )
                for m in range(nm):
                    nc.vector.tensor_mul(ot[:, m, :], pss[m], epi_sb)
            else:
                for m in range(nm):
                    ps = ps_mm.tile([P, O], f32, name="mm")
                    _emit_tile_mms(nc, ps, xt_t, slice(m * P, (m + 1) * P),
                                   w8lo, whi)
                    nc.vector.tensor_mul(ot[:, m, :], ps, epi_sb)
                    if gi == len(TGROUPS) - 1:
                        nc.sync.dma_start(
                            out=out[t0 + m * P : t0 + (m + 1) * P, :],
                            in_=ot[:, m, :],
                        )
            if gi != len(TGROUPS) - 1:
                nc.sync.dma_start(
                    out=out[t0 : t0 + ntok, :].rearrange("(m p) o -> p m o", p=P),
                    in_=ot,
                )


def build_nc():
    if "nc" in _CACHE:
        return _CACHE["nc"]
    nc = bacc.Bacc("TRN2", target_bir_lowering=False, debug=False,
                   num_devices=N_CORES)
    xt = nc.dram_tensor("xt", [D_IN, TOKENS], f8, kind="ExternalInput").ap()
    wt = nc.dram_tensor("wt", [P, NCH * O], f16, kind="ExternalInput").ap()
    a = nc.dram_tensor("a", [R, D_IN], f16, kind="ExternalInput").ap()
    at = nc.dram_tensor("at", [P, NCH * R], f16, kind="ExternalInput").ap()
    a2t = nc.dram_tensor("a2t", [P, NCH * R], f16, kind="ExternalInput").ap()
    b2t = nc.dram_tensor("b2t", [R, O], f16, kind="ExternalInput").ap()
    mag = nc.dram_tensor("mag", [P, O], f32, kind="ExternalInput").ap()
    cslo = nc.dram_tensor("cslo", [P, LO], f32, kind="ExternalInput").ap()
    epi = nc.dram_tensor("epi", [P, O], f32, kind="ExternalInput").ap()
    out = nc.dram_tensor("out", [TOKENS, O], f16, kind="ExternalOutput").ap()
    with tile.TileContext(nc) as tc:
        emit_kernel(nc, tc, xt, wt, a, at, a2t, b2t, mag, cslo, epi, out)
    nc.compile()
    _CACHE["nc"] = nc
    return nc


def prep_in_maps(x, lora_A_w, lora_B_w, base_w, magnitude):
    import ml_dtypes

    f8np = ml_dtypes.float8_e4m3   # IEEE e4m3 (max 240) = mybir float8e4
    x = np.asarray(x, np.float32)
    A = np.asarray(lora_A_w, np.float32)
    B = np.asarray(lora_B_w, np.float32)
    W = np.asarray(base_w, np.float32)
    mag = np.asarray(magnitude, np.float32)

    xt_np = np.ascontiguousarray((x.astype(np.float32) * SX).T.astype(f8np))
    a_np = np.ascontiguousarray(A.astype(np.float16))
    # A.T partition-major: at_dev[p, c*R + r] = A.T[c*128 + p, r]
    at_full = np.ascontiguousarray(a_np.T)                     # [4096, R]
    at_np = np.ascontiguousarray(
        at_full.reshape(NCH, P, R).transpose(1, 0, 2).reshape(P, NCH * R))
    a2t_full = np.ascontiguousarray(
        (2.0 * a_np.astype(np.float32)).astype(np.float16).T)  # [4096, R]
    a2t_np = np.ascontiguousarray(
        a2t_full.reshape(NCH, P, R).transpose(1, 0, 2).reshape(P, NCH * R))

    # host-side estimate of s - used ONLY for the column permutation and
    # the fp8 scale vectors (layout/marshaling decisions)
    lora_w = a_np.astype(np.float32)
    lora_w = B.astype(np.float16).astype(np.float32) @ lora_w
    comb = W + SCALING * lora_w
    s_h = mag / np.sqrt((comb * comb).sum(1))

    in_maps = []
    perms = []
    for c in range(N_CORES):
        sl = slice(c * O, (c + 1) * O)
        sc = s_h[sl]
        order = np.argsort(-np.abs(sc - 1.0), kind="stable")
        hi = np.sort(order[:N16])
        lo = np.sort(order[N16:])
        perm = np.concatenate([lo, hi])
        perms.append(perm)

        Wp = W[sl][perm]                       # [O, 4096] permuted rows
        sp = sc[perm]
        # W.T partition-major: wt_dev[p, c*O + o] = Wp.T[c*128 + p, o]
        wt_sh = np.ascontiguousarray(Wp.astype(np.float16).T)  # [4096, O]
        wt_dev = np.ascontiguousarray(
            wt_sh.reshape(NCH, P, O).transpose(1, 0, 2).reshape(P, NCH * O))

        # per-column fp8 scales from the host weff estimate (lo cols only)
        weff_sh = (sp[:, None] - 1.0) * Wp + \
            (SCALING * sp[:, None]) * lora_w[sl][perm]
        colmax = np.abs(weff_sh[:LO]).max(axis=1)
        cs = (CTARG / np.maximum(colmax, 1e-30)).astype(np.float32)
        epi_row = np.empty(O, np.float32)
        epi_row[:LO] = 1.0 / (SX * cs)
        epi_row[LO:] = 1.0 / SX

        in_maps.append({
            "xt": xt_np,
            "wt": wt_dev,
            "a": a_np,
            "at": at_np,
            "a2t": a2t_np,
            "b2t": np.ascontiguousarray(
                (SCALING * B[sl][perm]).astype(np.float16).T),
            "mag": np.ascontiguousarray(np.broadcast_to(
                mag[sl][perm].reshape(1, O).astype(np.float32), (P, O))),
            "cslo": np.ascontiguousarray(
                np.broadcast_to(cs.reshape(1, LO), (P, LO))),
            "epi": np.ascontiguousarray(
                np.broadcast_to(epi_row.reshape(1, O), (P, O))),
        })
    return in_maps, perms


def gather_out(res, perms):
    full = np.empty((TOKENS, D_OUT), np.float32)
    for c in range(N_CORES):
        shard = res.results[c]["out"].astype(np.float32)
        full[:, c * O + perms[c]] = shard
    return full


def kernel(x, lora_A_w, lora_B_w, base_w, magnitude):
    nc = build_nc()
    in_maps, perms = prep_in_maps(x, lora_A_w, lora_B_w, base_w, magnitude)
    res = run_bass_kernel_spmd(nc, in_maps, list(range(N_CORES)))
    return gather_out(res, perms)


# revision 7
# speedup vs baseline: 1.2577x; 1.0032x over previous
"""DoRA linear layer (nn_DoraLinearLayer) on 8 Trainium2 NeuronCores.

Math: out = (s-1)*(x @ W.T) + 2*s*((x @ A.T) @ B.T),
      s = magnitude / ||W + 2*B@A||_row  (stop-grad norm)

Fused single-GEMM form: out = x @ Weff.T with
      Weff.T = (s-1)*W.T + A.T @ (2B*s).T
The kernel computes s on-device (Gram/polarization identity on fp16
copies), builds Weff.T once, then streams x through the GEMM.

fp8 acceleration: the tensor engine runs float8e4 matmuls in DoubleRow
perf mode at 2x the fp16 rate (K=256 per instruction).  Weff columns
are split per-core by |s-1| (host-estimated, layout decision only):
the 416 columns with small |s-1| are quantized to fp8 with per-column
scales and streamed via DoubleRow against fp8 x; the 96 worst columns
stay fp16 (mixed fp8-lhsT x fp16-rhs matmuls are exact on TRN2).  Both
groups accumulate into one PSUM bank (fp16 group opens the bank with
start=True; fp8 group rides the pending-zero region).  A f32 epilogue
rescales per column and emits fp16.  Measured end-to-end rel err vs
the fp32 reference ~1.7e-2 (gate 2e-2); fp16 x / fp16 Weff gives
2.1e-4 but runs ~1.7x slower.

Sharding: column-parallel over out_features - core i owns rows
[i*512, (i+1)*512) of W/B/magnitude (host-permuted within the shard),
x and A replicated.  Host work is marshaling only: dtype casts,
transposes, the |s-1| permutation and fp8 scale vectors.
"""
import numpy as np

import concourse.bass as bass
import concourse.tile as tile
from concourse import bacc, mybir
from concourse.bass_utils import run_bass_kernel_spmd

N_CORES = 8
TOKENS, D_IN, D_OUT, R = 8192, 4096, 4096, 16
O = D_OUT // N_CORES          # 512 output features per core
P = 128                       # partitions
NCH = D_IN // P               # 32 contraction chunks
SCALING = 2.0                 # lora_alpha / r
N_WARM = 8                    # PE warmup matmuls
N16 = 64                      # columns kept in fp16 (largest |s-1|)
LO = O - N16                  # fp8 DoubleRow columns
SX = 32.0                     # fp8 scale for x
CTARG = 160.0                 # per-column fp8 target absmax for Weff

# token groups: first is chunk-major with 5 psum banks so the matmuls
# exactly saturate PE while trailing the weff feeder; last takes the rest
TGROUPS = [(0, 640, True)]
_t = 640
while _t + 512 <= TOKENS - 384:
    TGROUPS.append((_t, 512, False))
    _t += 512
TGROUPS.append((_t, TOKENS - _t, False))   # 384

f16 = mybir.dt.float16
f32 = mybir.dt.float32
bf16 = mybir.dt.bfloat16
f8 = mybir.dt.float8e4
DR = mybir.MatmulPerfMode.DoubleRow
Copy = mybir.ActivationFunctionType.Copy

_CACHE: dict = {}


def _emit_tile_mms(nc, ps, xt_t, msl, w8lo, whi):
    """All matmuls for one 128-token tile into psum `ps` ([128, 512]).

    fp16 group (cols LO:) opens the bank (start=True zeroes the whole
    2KB zero-region); the fp8 DoubleRow group (cols :LO) accumulates
    onto the pending-zero bytes, pairing chunks (c-1, c) at odd c.
    """
    psA, psB = ps[:, :LO], ps[:, LO:]
    for c in range(NCH):
        nc.tensor.matmul(
            psB, lhsT=xt_t[:, c, msl], rhs=whi[:, c, :],
            start=(c == 0), stop=(c == NCH - 1), skip_group_check=True,
        )
        if c % 2 == 1:
            nc.tensor.matmul(
                psA, lhsT=xt_t[:, c - 1 : c + 1, msl],
                rhs=w8lo[:, c - 1 : c + 1, :],
                start=False, stop=(c == NCH - 1), perf_mode=DR,
                skip_group_check=True,
            )


def emit_kernel(nc, tc, xt, wt, a, at, a2t, b2t, mag, cslo, epi, out):
    """Emit the per-core program. All DRAM APs are per-core shapes."""
    from contextlib import ExitStack

    with ExitStack() as ctx:
        singles = ctx.enter_context(tc.tile_pool(name="singles", bufs=1))
        setup = ctx.enter_context(tc.tile_pool(name="setup", bufs=3))
        # 8 PSUM banks: scr(2) + lws(1) + mm(5)
        ps_scr = ctx.enter_context(tc.tile_pool(name="ps_scr", bufs=2, space="PSUM"))
        ps_lws = ctx.enter_context(tc.tile_pool(name="ps_lws", bufs=1, space="PSUM"))
        ps_mm = ctx.enter_context(tc.tile_pool(name="ps_mm", bufs=5, space="PSUM"))
        wtpool = ctx.enter_context(tc.tile_pool(name="wtpool", bufs=1))
        xpool = ctx.enter_context(tc.tile_pool(name="xpool", bufs=2))
        x0pool = ctx.enter_context(tc.tile_pool(name="x0pool", bufs=1))
        opool = ctx.enter_context(tc.tile_pool(name="opool", bufs=2))

        # ---- at/a2t ride the fast SP ring ahead of wt; the rest go via
        # GPSIMD SWDGE ordered by first use
        at_sb = singles.tile([P, NCH, R], f16)
        nc.sync.dma_start(out=at_sb, in_=at.rearrange("p (c r) -> p c r", r=R))
        a2t_sb = singles.tile([P, NCH, R], f16)
        nc.sync.dma_start(out=a2t_sb, in_=a2t.rearrange("p (c r) -> p c r", r=R))
        b2t_sb = singles.tile([R, O], f16)
        nc.gpsimd.dma_start(out=b2t_sb, in_=b2t)
        a_sb = singles.tile([R, D_IN], f16)
        nc.gpsimd.dma_start(out=a_sb, in_=a)
        mag_sb = singles.tile([P, O], f32)
        nc.gpsimd.dma_start(out=mag_sb, in_=mag)
        cslo_sb = singles.tile([P, LO], f16)
        nc.gpsimd.dma_start(out=cslo_sb, in_=cslo)
        epi_sb = singles.tile([P, O], f32)
        nc.gpsimd.dma_start(out=epi_sb, in_=epi)

        ones_col = singles.tile([P, 1], bf16)
        nc.vector.memset(ones_col, 1.0)
        ones16_col = singles.tile([R, 1], f16)
        nc.vector.memset(ones16_col, 1.0)
        ones_row16 = singles.tile([1, P], f16)
        nc.vector.memset(ones_row16, 1.0)
        ones_row32 = singles.tile([1, P], f32)
        nc.vector.memset(ones_row32, 1.0)

        # pre-warm the ACT Sqrt table so the s-chain doesn't pay the load
        sqrt_warm = singles.tile([1, 1], f32)
        nc.vector.memset(sqrt_warm, 1.0)
        sqrt_warm2 = singles.tile([1, 1], f32)
        nc.scalar.sqrt(sqrt_warm2, sqrt_warm)

        # ---- PE warmup: cheap matmuls to lift the HAM clock gate ----
        warm_row = singles.tile([1, O], f16)
        nc.vector.memset(warm_row, 0.5)
        warm8 = singles.tile([P, 2, 64], f8)
        nc.vector.memset(warm8, 0.25)
        warm8r = singles.tile([P, 2, O], f8)
        nc.vector.memset(warm8r, 0.25)
        warm_ps = ps_scr.tile([P, O], f32, name="scr")
        for i in range(N_WARM):
            if i % 2 == 0:
                nc.tensor.matmul(warm_ps, lhsT=ones_row16, rhs=warm_row,
                                 start=True, stop=True)
            else:
                nc.tensor.matmul(warm_ps[:64, :], lhsT=warm8, rhs=warm8r,
                                 start=True, stop=True, perf_mode=DR)

        # ---- W.T: host-prearranged [p, c, o]; wave DMAs of chunks ----
        wt_sb = wtpool.tile([P, NCH, O], f16, name="wt16")
        wt_r = wt.rearrange("p (c o) -> p c o", o=O)
        wt_dmas = []
        wave_edges = [0, 1, 2, 4, 7, 11, 16, 23, NCH]
        for w in range(len(wave_edges) - 1):
            lo_, hi_ = wave_edges[w], wave_edges[w + 1]
            q = nc.sync if w % 2 == 0 else nc.scalar
            wt_dmas.append(
                q.dma_start(out=wt_sb[:, lo_:hi_, :],
                            in_=wt_r[:, lo_:hi_, :]))
        wt_t = [wt_sb[:, c, :] for c in range(NCH)]

        # ---- phase A: n2 = ||W||^2_col + sum_r B2T o (2A@W.T + G@B2T) ----
        g_ps = ps_scr.tile([R, R], f32, name="scr")
        for c in range(NCH):
            nc.tensor.matmul(
                g_ps, lhsT=at_sb[:, c, :], rhs=at_sb[:, c, :],
                start=(c == 0), stop=(c == NCH - 1),
            )
        g_sb = singles.tile([R, R], f16)
        nc.scalar.activation(g_sb, g_ps, Copy)

        h_ps = ps_scr.tile([R, O], f32, name="scr")
        n2_ps = ps_scr.tile([1, O], f32, name="scr")
        # sq production runs ahead on DVE/ACT so the PE h+n2 chain
        # never waits; 6-deep sq pool gives the lookahead window
        sqs = []
        for c in range(NCH):
            sq = setup.tile([P, O], bf16, name="sq", bufs=6)
            if c % 2 == 0:
                nc.vector.tensor_mul(sq, wt_t[c], wt_t[c])
            else:
                nc.scalar.square(sq, wt_t[c])
            sqs.append(sq)
        for c in range(NCH):
            nc.tensor.matmul(
                h_ps, lhsT=a2t_sb[:, c, :], rhs=wt_t[c],
                start=(c == 0), stop=(c == NCH - 1),
            )
            if c == 0:
                nc.tensor.matmul(h_ps, lhsT=g_sb, rhs=b2t_sb,
                                 start=False, stop=False)
            nc.tensor.matmul(
                n2_ps, lhsT=ones_col, rhs=sqs[c],
                start=(c == 0), stop=False,
            )
        hterm = singles.tile([R, O], f16)
        nc.vector.tensor_mul(hterm, b2t_sb, h_ps)
        nc.tensor.matmul(n2_ps, lhsT=ones16_col, rhs=hterm,
                         start=False, stop=True)

        # ---- prefetch x.T for the first two token groups ----
        t0, ntok0, _ = TGROUPS[0]
        xt0 = x0pool.tile([P, NCH, ntok0], f8, name="xt0")
        xg0 = xt[:, t0 : t0 + ntok0].rearrange("(c p) t -> c p t", p=P)
        from concourse.tile_rust import add_dep_helper
        for c in range(NCH):
            dma = nc.sync.dma_start(out=xt0[:, c, :], in_=xg0[c])
            if c == 0:
                # keep x prefetch off the HBM bus until W.T has landed -
                # the wt waves gate the whole setup critical path
                for wd in wt_dmas:
                    add_dep_helper(dma.ins, wd.ins, True, "x prefetch after wt")
        t1, ntok1, _ = TGROUPS[1]
        xt1 = xpool.tile([P, NCH, ntok1], f8, name="xt")
        xg1 = xt[:, t1 : t1 + ntok1].rearrange("(c p) t -> c p t", p=P)
        nc.sync.dma_start(out=xt1, in_=xg1.rearrange("c p t -> p c t"))
        xt_pre = {0: xt0, 1: xt1}

        # ---- s = mag / sqrt(n2); broadcast; sm1 = s - 1 ----
        nrm = singles.tile([1, O], f32)
        nc.scalar.sqrt(nrm, n2_ps)
        rn = singles.tile([1, O], f32)
        nc.vector.reciprocal_approx_fast(out=rn, in_=nrm)
        brn_ps = ps_scr.tile([P, O], f32, name="scr")
        nc.tensor.matmul(brn_ps, lhsT=ones_row32, rhs=rn, start=True, stop=True)
        s_bc = singles.tile([P, O], f32)
        nc.vector.tensor_mul(s_bc, mag_sb, brn_ps)
        bcast_sm1 = singles.tile([P, O], f16)
        nc.vector.tensor_scalar_add(bcast_sm1, s_bc, -1.0)
        # (2B*s).T = (2B).T with columns scaled by s
        b2st_sb = singles.tile([R, O], f16)
        nc.vector.tensor_mul(b2st_sb, b2t_sb, s_bc[:R, :])

        # ---- phase B: Weff.T = (s-1)*W.T + A.T @ (2B*s).T; quantize the
        # lo columns to fp8 with per-column scales, keep hi columns fp16
        w8lo = singles.tile([P, NCH, LO], f8, name="w8lo")
        whi = singles.tile([P, NCH, N16], f16, name="whi")
        for c in range(NCH):
            lws_ps = ps_lws.tile([P, O], f32, name="lws")
            nc.tensor.matmul(
                lws_ps, lhsT=a_sb[:, c * P : (c + 1) * P], rhs=b2st_sb,
                start=True, stop=True,
            )
            tmp = setup.tile([P, O], f16, name="tmp")
            nc.vector.tensor_mul(tmp, wt_t[c], bcast_sm1)
            nc.vector.tensor_add(whi[:, c, :], tmp[:, LO:], lws_ps[:, LO:])
            wlo16 = setup.tile([P, LO], f16, name="wlo16")
            nc.vector.tensor_add(wlo16, tmp[:, :LO], lws_ps[:, :LO])
            nc.vector.tensor_mul(w8lo[:, c, :], wlo16, cslo_sb)

        # ---- main: out = x @ Weff.T, streamed over token groups ----
        for gi, (t0, ntok, chunk_major) in enumerate(TGROUPS):
            nm = ntok // P
            if gi in xt_pre:
                xt_t = xt_pre[gi]
            else:
                xt_t = xpool.tile([P, NCH, ntok], f8, name="xt")
                xg = xt[:, t0 : t0 + ntok].rearrange("(c p) t -> c p t", p=P)
                nc.sync.dma_start(out=xt_t, in_=xg.rearrange("c p t -> p c t"))
            ot = opool.tile([P, nm, O], f16, name="ot")
            if chunk_major:
                # consume each weff chunk nm x as soon as it lands
                pss = [ps_mm.tile([P, O], f32, name="mm") for _ in range(nm)]
                for c in range(NCH):
                    for m in range(nm):
                        msl = slice(m * P, (m + 1) * P)
                        nc.tensor.matmul(
                            pss[m][:, LO:], lhsT=xt_t[:, c, msl],
                            rhs=whi[:, c, :],
                            start=(c == 0), stop=(c == NCH - 1),
                            skip_group_check=True,
                        )
                    if c % 2 == 1:
                        for m in range(nm):
                            msl = slice(m * P, (m + 1) * P)
                            nc.tensor.matmul(
                                pss[m][:, :LO],
                                lhsT=xt_t[:, c - 1 : c + 1, msl],
                                rhs=w8lo[:, c - 1 : c + 1, :],
                                start=False, stop=(c == NCH - 1),
                                perf_mode=DR, skip_group_check=True,
                            )
                for m in range(nm):
                    nc.vector.tensor_mul(ot[:, m, :], pss[m], epi_sb)
            else:
                for m in range(nm):
                    ps = ps_mm.tile([P, O], f32, name="mm")
                    _emit_tile_mms(nc, ps, xt_t, slice(m * P, (m + 1) * P),
                                   w8lo, whi)
                    nc.vector.tensor_mul(ot[:, m, :], ps, epi_sb)
                    if gi == len(TGROUPS) - 1:
                        nc.sync.dma_start(
                            out=out[t0 + m * P : t0 + (m + 1) * P, :],
                            in_=ot[:, m, :],
                        )
            if gi != len(TGROUPS) - 1:
                nc.sync.dma_start(
                    out=out[t0 : t0 + ntok, :].rearrange("(m p) o -> p m o", p=P),
                    in_=ot,
                )


def build_nc():
    if "nc" in _CACHE:
        return _CACHE["nc"]
    nc = bacc.Bacc("TRN2", target_bir_lowering=False, debug=False,
                   num_devices=N_CORES)
    xt = nc.dram_tensor("xt", [D_IN, TOKENS], f8, kind="ExternalInput").ap()
    wt = nc.dram_tensor("wt", [P, NCH * O], f16, kind="ExternalInput").ap()
    a = nc.dram_tensor("a", [R, D_IN], f16, kind="ExternalInput").ap()
    at = nc.dram_tensor("at", [P, NCH * R], f16, kind="ExternalInput").ap()
    a2t = nc.dram_tensor("a2t", [P, NCH * R], f16, kind="ExternalInput").ap()
    b2t = nc.dram_tensor("b2t", [R, O], f16, kind="ExternalInput").ap()
    mag = nc.dram_tensor("mag", [P, O], f32, kind="ExternalInput").ap()
    cslo = nc.dram_tensor("cslo", [P, LO], f16, kind="ExternalInput").ap()
    epi = nc.dram_tensor("epi", [P, O], f32, kind="ExternalInput").ap()
    out = nc.dram_tensor("out", [TOKENS, O], f16, kind="ExternalOutput").ap()
    with tile.TileContext(nc) as tc:
        emit_kernel(nc, tc, xt, wt, a, at, a2t, b2t, mag, cslo, epi, out)
    nc.compile()
    _CACHE["nc"] = nc
    return nc


def prep_in_maps(x, lora_A_w, lora_B_w, base_w, magnitude):
    import ml_dtypes

    f8np = ml_dtypes.float8_e4m3   # IEEE e4m3 (max 240) = mybir float8e4
    x = np.asarray(x, np.float32)
    A = np.asarray(lora_A_w, np.float32)
    B = np.asarray(lora_B_w, np.float32)
    W = np.asarray(base_w, np.float32)
    mag = np.asarray(magnitude, np.float32)

    xt_np = np.ascontiguousarray((x.astype(np.float32) * SX).T.astype(f8np))
    a_np = np.ascontiguousarray(A.astype(np.float16))
    # A.T partition-major: at_dev[p, c*R + r] = A.T[c*128 + p, r]
    at_full = np.ascontiguousarray(a_np.T)                     # [4096, R]
    at_np = np.ascontiguousarray(
        at_full.reshape(NCH, P, R).transpose(1, 0, 2).reshape(P, NCH * R))
    a2t_full = np.ascontiguousarray(
        (2.0 * a_np.astype(np.float32)).astype(np.float16).T)  # [4096, R]
    a2t_np = np.ascontiguousarray(
        a2t_full.reshape(NCH, P, R).transpose(1, 0, 2).reshape(P, NCH * R))

    # host-side estimate of s - used ONLY for the column permutation and
    # the fp8 scale vectors (layout/marshaling decisions)
    lora_w = a_np.astype(np.float32)
    lora_w = B.astype(np.float16).astype(np.float32) @ lora_w
    comb = W + SCALING * lora_w
    s_h = mag / np.sqrt((comb * comb).sum(1))

    in_maps = []
    perms = []
    for c in range(N_CORES):
        sl = slice(c * O, (c + 1) * O)
        sc = s_h[sl]
        order = np.argsort(-np.abs(sc - 1.0), kind="stable")
        hi = np.sort(order[:N16])
        lo = np.sort(order[N16:])
        perm = np.concatenate([lo, hi])
        perms.append(perm)

        Wp = W[sl][perm]                       # [O, 4096] permuted rows
        sp = sc[perm]
        # W.T partition-major: wt_dev[p, c*O + o] = Wp.T[c*128 + p, o]
        wt_sh = np.ascontiguousarray(Wp.astype(np.float16).T)  # [4096, O]
        wt_dev = np.ascontiguousarray(
            wt_sh.reshape(NCH, P, O).transpose(1, 0, 2).reshape(P, NCH * O))

        # per-column fp8 scales from the host weff estimate (lo cols only)
        weff_sh = (sp[:, None] - 1.0) * Wp + \
            (SCALING * sp[:, None]) * lora_w[sl][perm]
        colmax = np.abs(weff_sh[:LO]).max(axis=1)
        cs = (CTARG / np.maximum(colmax, 1e-30)).astype(np.float16)
        epi_row = np.empty(O, np.float32)
        epi_row[:LO] = 1.0 / (SX * cs.astype(np.float32))
        epi_row[LO:] = 1.0 / SX

        in_maps.append({
            "xt": xt_np,
            "wt": wt_dev,
            "a": a_np,
            "at": at_np,
            "a2t": a2t_np,
            "b2t": np.ascontiguousarray(
                (SCALING * B[sl][perm]).astype(np.float16).T),
            "mag": np.ascontiguousarray(np.broadcast_to(
                mag[sl][perm].reshape(1, O).astype(np.float32), (P, O))),
            "cslo": np.ascontiguousarray(
                np.broadcast_to(cs.reshape(1, LO), (P, LO))),
            "epi": np.ascontiguousarray(
                np.broadcast_to(epi_row.reshape(1, O), (P, O))),
        })
    return in_maps, perms


def gather_out(res, perms):
    full = np.empty((TOKENS, D_OUT), np.float32)
    for c in range(N_CORES):
        shard = res.results[c]["out"].astype(np.float32)
        full[:, c * O + perms[c]] = shard
    return full


def kernel(x, lora_A_w, lora_B_w, base_w, magnitude):
    nc = build_nc()
    in_maps, perms = prep_in_maps(x, lora_A_w, lora_B_w, base_w, magnitude)
    res = run_bass_kernel_spmd(nc, in_maps, list(range(N_CORES)))
    return gather_out(res, perms)
